# revision 4
# baseline (speedup 1.0000x reference)
"""V5: 2-layer LSTMP on 8 TRN2 cores — 8-rank AllGather exchange.

Key change vs kv3: back to REDUNDANT groups (each core computes gates for
the full batch 32; the two 4-core groups do identical work). The per-step
m-exchange becomes a single 8-rank AllGather (floor ~4.6us, vs ~10.6us
for dual 4-rank AG / ~20us for 4-rank AllReduce). Because ranks 4-7
duplicate ranks 0-3, every core uniformly sums gathered blocks 0-3 —
fully SPMD-uniform, no per-core specialization.

Also: zx rings in bf16 (mixed-dtype DVE add), chunks of CHT=8 steps.
"""
import numpy as np
import ml_dtypes

import concourse.bass as bass
import concourse.bacc as bacc
import concourse.mybir as mybir
import concourse.tile as tile
import concourse.bass_utils as bass_utils

F32 = mybir.dt.float32
BF16 = mybir.dt.bfloat16

B = 32
FEAT = 640
H = 2048
P = 640
N_CORES = 8
NKT = 5
REF_GATES = (0, 2, 3, 1)  # device gate order (i, f, o, j) -> reference index
FORGET_BIAS = 1.0
TPC = 4

HS = H // TPC        # 512
NHB = HS // 128      # 4
NCB = 4 * NHB        # 16
ZW = NCB * B         # 512
CW = NHB * B         # 128
SW = 3 * CW          # 384
MW = NKT * B         # 160
CHT = 8              # timesteps per zx chunk
CH = CHT * B         # 256 moving cols per chunk matmul
DELAY = 2 * CHT      # layer-2 lag in steps
RING_BUFS = 3


def _shard_weights(W, b, Pm, rank):
    cols = []
    for cb in range(NCB):
        g = REF_GATES[cb // NHB]
        hb = cb % NHB
        base = g * H + rank * HS + hb * 128
        cols.append(np.arange(base, base + 128))
    cols = np.concatenate(cols)
    Wk = W[:, cols]
    Wx = Wk[:FEAT].reshape(NKT, 128, NCB * 128)
    Wm = Wk[FEAT:].reshape(NKT, 128, NCB * 128)
    bk = b[cols].copy()
    for cb in range(NCB):
        if REF_GATES[cb // NHB] == 2:
            bk[cb * 128:(cb + 1) * 128] += FORGET_BIAS
    bias = np.ascontiguousarray(bk.reshape(NCB, 128).T)
    Pk = Pm[rank * HS:(rank + 1) * HS].reshape(NHB, 128, P)
    return Wx, Wm, bias, Pk


def _bf16(x):
    return x.astype(ml_dtypes.bfloat16)


def _prep_inputs(inputs, T):
    x = np.asarray(inputs["x"], np.float32)[:, :T]
    xT = _bf16(np.ascontiguousarray(
        x.transpose(2, 1, 0).reshape(NKT, 128, T * B)))
    in_maps = []
    for k in range(N_CORES):
        r = k % TPC
        m = {"xT": xT}
        for li, (W, b, Pm) in enumerate(
            [(inputs["W0"], inputs["b0"], inputs["P0"]),
             (inputs["W1"], inputs["b1"], inputs["P1"])]
        ):
            Wx, Wm, bias, Pk = _shard_weights(
                np.asarray(W, np.float32), np.asarray(b, np.float32),
                np.asarray(Pm, np.float32), r
            )
            m[f"Wx{li}"] = _bf16(Wx)
            m[f"Wm{li}"] = _bf16(Wm)
            m[f"bias{li}"] = bias
            m[f"P{li}"] = _bf16(Pk)
        in_maps.append(m)
    return in_maps


def _unshard_out(res, T):
    o = np.asarray(res.results[0]["outT"], ml_dtypes.bfloat16)
    o = o.astype(np.float32).reshape(T, 128, NKT, B)
    return np.ascontiguousarray(o.transpose(3, 0, 2, 1).reshape(B, T, P))


def _build(T, repeat=1):
    BT = B * T
    n_chunks = T // CHT
    groups = [list(range(N_CORES))]

    nc = bacc.Bacc(
        "TRN2",
        target_bir_lowering=False,
        debug=False,
        enable_asserts=True,
        num_devices=N_CORES,
    )
    xT_d = nc.dram_tensor("xT", [NKT, 128, BT], BF16, kind="ExternalInput")
    Wx_d, Wm_d, bias_d, P_d = [], [], [], []
    for li in range(2):
        Wx_d.append(nc.dram_tensor(f"Wx{li}", [NKT, 128, NCB * 128], BF16, kind="ExternalInput"))
        Wm_d.append(nc.dram_tensor(f"Wm{li}", [NKT, 128, NCB * 128], BF16, kind="ExternalInput"))
        bias_d.append(nc.dram_tensor(f"bias{li}", [128, NCB], F32, kind="ExternalInput"))
        P_d.append(nc.dram_tensor(f"P{li}", [NHB, 128, P], BF16, kind="ExternalInput"))
    outT_d = nc.dram_tensor("outT", [T, 128, MW], BF16, kind="ExternalOutput")

    with tile.TileContext(nc) as tc:
        with (
            tc.tile_pool(name="wpool", bufs=1) as wpool,
            tc.tile_pool(name="spool", bufs=2) as spool,
            tc.tile_pool(name="rhspool", bufs=6) as rhspool,
            tc.tile_pool(name="mtpool", bufs=3) as mtpool,
            tc.tile_pool(name="pers", bufs=1) as pers,
            tc.tile_pool(name="psum", bufs=2, space="PSUM") as psum,
            tc.tile_pool(name="psz", bufs=2, space="PSUM") as psz,
            tc.tile_pool(name="ccpool", bufs=4, space="DRAM") as ccpool,
        ):
            bias_sb = [pers.tile([128, NCB], F32, tag=f"bias{li}", name=f"bias_sb{li}") for li in range(2)]
            for li in range(2):
                nc.sync.dma_start(bias_sb[li][:], bias_d[li][:])

            Wm_sb, P_sb, Wx_sb = [], [], []
            for li in range(2):
                w = wpool.tile([128, NKT * NCB * 128], BF16, tag=f"wm{li}", name=f"wm_sb{li}")
                for kt in range(NKT):
                    nc.sync.dma_start(w[:, kt * NCB * 128:(kt + 1) * NCB * 128], Wm_d[li][kt])
                Wm_sb.append(w)
                p = wpool.tile([128, NHB * P], BF16, tag=f"p{li}", name=f"p_sb{li}")
                for hb in range(NHB):
                    nc.sync.dma_start(p[:, hb * P:(hb + 1) * P], P_d[li][hb])
                P_sb.append(p)
                w2 = wpool.tile([128, NKT * NCB * 128], BF16, tag=f"wx{li}", name=f"wx_sb{li}")
                for kf in range(NKT):
                    nc.sync.dma_start(w2[:, kf * NCB * 128:(kf + 1) * NCB * 128], Wx_d[li][kf])
                Wx_sb.append(w2)

            # zx rings: bf16, RING_BUFS chunk buffers per layer, CHT steps each
            rings = [
                [pers.tile([128, CHT * ZW], BF16, tag=f"ring{li}_{rb}",
                           name=f"ring{li}_{rb}") for rb in range(RING_BUFS)]
                for li in range(2)
            ]
            # layer-1 zx rhs staging (layer-0 m outputs), 3 parity buffers
            hstage = [pers.tile([128, NKT * CH], BF16, tag=f"hstage{par}",
                                name=f"hstage{par}") for par in range(3)]

            def zx_chunk(li, ci, rhs_tiles):
                ring = rings[li][ci % RING_BUFS]
                for cb in range(NCB):
                    zp = psum.tile([128, CH], F32, tag="zxps", name=f"zxps_{li}_{ci}_{cb}")
                    for kf in range(NKT):
                        nc.tensor.matmul(
                            zp[:],
                            Wx_sb[li][:, kf * NCB * 128 + cb * 128: kf * NCB * 128 + cb * 128 + 128],
                            rhs_tiles[kf],
                            start=(kf == 0),
                            stop=(kf == NKT - 1),
                        )
                    # scatter [CHT t x B b] into ring at (t*ZW + cb*B + b)
                    dst = ring[:].rearrange(
                        "p (t cb b) -> p t cb b", t=CHT, b=B)[:, :, cb, :]
                    nc.scalar.activation(
                        dst, zp[:].rearrange("p (t b) -> p t b", b=B),
                        mybir.ActivationFunctionType.Identity,
                        bias=bias_sb[li][:, cb:cb + 1],
                    )

            def x_chunk(li, ci):
                c0 = ci * CH
                rhs = []
                for kf in range(NKT):
                    rt = rhspool.tile([128, CH], BF16, tag="rhs", name=f"rhs_{li}_{ci}_{kf}")
                    nc.scalar.dma_start(rt[:], xT_d[kf, :, c0:c0 + CH])
                    rhs.append(rt[:])
                zx_chunk(li, ci, rhs)

            def h_chunk(ci):
                par = hstage[ci % 3]
                rhs = [par[:, kf * CH:(kf + 1) * CH] for kf in range(NKT)]
                zx_chunk(1, ci, rhs)

            def make_state(li):
                c_sb = pers.tile([128, CW], F32, tag=f"c{li}", name=f"c_sb{li}")
                nc.vector.memset(c_sb[:], 0.0)
                mT = mtpool.tile([128, MW], BF16, tag=f"mT{li}", name=f"mT_{li}_init")
                nc.vector.memset(mT[:], 0.0)
                return {"c": c_sb, "mT": mT}

            def step(li, t, st):
                z_ps = psz.tile([128, ZW], F32, tag=f"zps{li}", name=f"zps_{li}_{t}", bufs=1)
                for cb in range(NCB):
                    for kt in range(NKT):
                        nc.tensor.matmul(
                            z_ps[:, B * cb:B * cb + B],
                            Wm_sb[li][:, kt * NCB * 128 + cb * 128: kt * NCB * 128 + cb * 128 + 128],
                            st["mT"][:, B * kt:B * kt + B],
                            start=(kt == 0),
                            stop=(kt == NKT - 1),
                        )
                ring = rings[li][(t // CHT) % RING_BUFS]
                zx_t = ring[:, (t % CHT) * ZW:(t % CHT) * ZW + ZW]
                z_sb = spool.tile([128, ZW], F32, tag=f"z{li}", name=f"z_{li}_{t}")
                nc.vector.tensor_add(z_sb[:], z_ps[:], zx_t)
                sig = spool.tile([128, SW], F32, tag=f"sig{li}", name=f"sig_{li}_{t}")
                nc.scalar.activation(sig[:], z_sb[:, 0:SW], mybir.ActivationFunctionType.Sigmoid)
                tj = spool.tile([128, CW], F32, tag=f"tj{li}", name=f"tj_{li}_{t}")
                nc.scalar.activation(tj[:], z_sb[:, SW:SW + CW], mybir.ActivationFunctionType.Tanh)
                t1 = spool.tile([128, CW], F32, tag=f"t1{li}", name=f"t1_{li}_{t}")
                nc.vector.tensor_mul(t1[:], sig[:, CW:2 * CW], st["c"][:])
                t2 = spool.tile([128, CW], F32, tag=f"t2{li}", name=f"t2_{li}_{t}")
                nc.vector.tensor_mul(t2[:], sig[:, 0:CW], tj[:])
                nc.vector.tensor_add(st["c"][:], t1[:], t2[:])
                tc_ = spool.tile([128, CW], F32, tag=f"tc{li}", name=f"tc_{li}_{t}")
                nc.scalar.activation(tc_[:], st["c"][:], mybir.ActivationFunctionType.Tanh)
                h_sb = spool.tile([128, CW], BF16, tag=f"h{li}", name=f"h_{li}_{t}")
                nc.vector.tensor_mul(h_sb[:], sig[:, 2 * CW:3 * CW], tc_[:])

                mp_ps = psz.tile([128, MW], F32, tag=f"mpps{li}", name=f"mpps_{li}_{t}", bufs=1)
                for mt in range(NKT):
                    for hb in range(NHB):
                        nc.tensor.matmul(
                            mp_ps[:, B * mt:B * mt + B],
                            P_sb[li][:, hb * P + mt * 128: hb * P + mt * 128 + 128],
                            h_sb[:, B * hb:B * hb + B],
                            start=(hb == 0),
                            stop=(hb == NHB - 1),
                        )
                mp_sb = spool.tile([128, MW], BF16, tag=f"mp{li}", name=f"mp_{li}_{t}")
                nc.vector.tensor_copy(mp_sb[:], mp_ps[:])

                cc_in = ccpool.tile([128, MW], BF16, tag=f"ccin{li}", name=f"ccin_{li}_{t}")
                nc.sync.dma_start(cc_in[:], mp_sb[:])
                ag_out = ccpool.tile([N_CORES, 128, MW], BF16, tag=f"agout{li}",
                                     name=f"agout_{li}_{t}")
                nc.gpsimd.collective_compute(
                    "AllGather",
                    mybir.AluOpType.bypass,
                    replica_groups=groups,
                    ins=[cc_in[:].opt()],
                    outs=[ag_out[:].opt()],
                )
                # ranks 4-7 duplicate ranks 0-3: load + sum blocks 0-3 only
                ag_sb = mtpool.tile([128, TPC * MW], BF16, tag=f"agsb{li}",
                                    name=f"agsb_{li}_{t}")
                nc.sync.dma_start(
                    ag_sb[:].rearrange("p (r i) -> p r i", r=TPC),
                    ag_out[0:TPC].rearrange("r p i -> p r i"),
                )
                s01 = spool.tile([128, MW], F32, tag=f"s01{li}", name=f"s01_{li}_{t}")
                nc.vector.tensor_add(s01[:], ag_sb[:, 0:MW], ag_sb[:, MW:2 * MW])
                s23 = spool.tile([128, MW], F32, tag=f"s23{li}", name=f"s23_{li}_{t}")
                nc.vector.tensor_add(s23[:], ag_sb[:, 2 * MW:3 * MW], ag_sb[:, 3 * MW:4 * MW])
                mT = mtpool.tile([128, MW], BF16, tag=f"mT{li}", name=f"mT_{li}_{t}")
                nc.vector.tensor_add(mT[:], s01[:], s23[:])
                st["mT"] = mT

                if li == 0:
                    # stage reduced m for layer-1's zx chunk
                    par = hstage[(t // CHT) % 3]
                    dst = par[:].rearrange(
                        "p (kf t b) -> p kf t b", kf=NKT, b=B)[:, :, t % CHT, :]
                    nc.vector.tensor_copy(
                        dst, mT[:].rearrange("p (kf b) -> p kf b", b=B))
                else:
                    nc.sync.dma_start(outT_d[t], mT[:])

            for rep in range(repeat):
                st0 = make_state(0)
                st1 = make_state(1)
                x_chunk(0, 0)
                x_chunk(0, 1)
                for tt in range(T + DELAY):
                    if tt < T:
                        step(0, tt, st0)
                    if tt >= DELAY:
                        step(1, tt - DELAY, st1)
                    if (tt + 1) % CHT == 0:
                        k = (tt + 1) // CHT
                        if k + 1 < n_chunks:
                            x_chunk(0, k + 1)
                        if k <= n_chunks:
                            h_chunk(k - 1)

    nc.compile()
    return nc


_CACHE = {}


def kernel(**inputs) -> np.ndarray:
    T = np.asarray(inputs["x"]).shape[1]
    if T not in _CACHE:
        _CACHE[T] = _build(T)
    nc = _CACHE[T]
    in_maps = _prep_inputs(inputs, T)
    last_err = None
    for _ in range(2):
        try:
            res = bass_utils.run_bass_kernel_spmd(
                nc, in_maps, core_ids=list(range(N_CORES))
            )
            return _unshard_out(res, T).astype(np.float32)
        except Exception as e:  # noqa: BLE001
            last_err = e
    raise last_err
